# revision 1
# baseline (speedup 1.0000x reference)
"""MoE FFN (top-2 of 8 experts, SwiGLU) for 8 Trainium2 NeuronCores.

Strategy: expert parallelism. The router (tiny [T,H]@[H,E] matmul + softmax +
top-2) runs on host as part of sharding; tokens are dispatched ("alltoall by
routing decision") to the core owning their expert. Each core runs a dense
SwiGLU FFN over its gathered tokens in bf16 (fp32 PSUM accumulation), in a
feature-on-partition / token-on-free-dim layout so no on-device transposes are
needed and every weight byte is DMA'd exactly once, as a handful of large
contiguous transfers. The host applies the combine weights and scatter-adds
the per-expert outputs into the full output.

Per-core device program (expert e), with nht = H/128 h-tiles, f-chunks of
FCH columns (nft f-tiles each):
  g_T[f, t] = sum_i w1[h_i, f]^T @ x_T[h_i, t]        (PSUM accum over h-tiles)
  u_T[f, t] likewise with w2
  h_T[f, t] = silu(g_T + b1) * (u_T + b2)             (ACT + DVE, -> bf16)
  y_T[h, t] = sum_f w3[f, h]^T @ h_T[f, t] + b3       (PSUM accum per f-chunk,
                                                       accumulated in SBUF f32)
Weights stream through SBUF one f-chunk at a time; tokens/outputs are SBUF-
resident. Every matmul has a 128-row stationary operand in natural layout and
a [128, block] moving operand, so the PE runs back-to-back at stream rate.
"""

import numpy as np
import ml_dtypes

E = 8       # experts == cores
K = 2       # top-k
H = 1024    # hidden
F = 4096    # ffn dim
BLK = 512   # max tokens per block (moving free dim of every matmul)
FCH = 512   # f-chunk size (weight streaming granularity); FCH % 128 == 0

NHT = H // 128    # h-tiles
NFCH = F // FCH   # f-chunks
NFT = FCH // 128  # f-tiles per chunk

_BF16 = ml_dtypes.bfloat16

_kernel_cache: dict[object, object] = {}
_last_in_maps = None


def _blocks_for(max_n: int):
    """Token-block sizes covering max_n tokens: full 512-blocks plus a small
    tail block, so padded capacity hugs the real max expert load."""
    max_n = max(max_n, 16)
    nfull, rem = divmod(max_n, BLK)
    rem = (rem + 7) // 8 * 8  # keep DMA rows 16B-aligned
    sizes = [BLK] * nfull + ([rem] if rem else [])
    blocks = []
    off = 0
    for sz in sizes:
        blocks.append((off, sz))
        off += sz
    return blocks, off


def _build(blocks, use_b2: bool):
    """Build the per-core Bass/Tile program for the given token blocks."""
    import concourse.bass as bass  # noqa: F401
    import concourse.tile as tile
    from concourse import bacc, mybir

    bf16 = mybir.dt.bfloat16
    f32 = mybir.dt.float32
    AF = mybir.ActivationFunctionType

    cap = sum(sz for _, sz in blocks)

    nc = bacc.Bacc("TRN2", target_bir_lowering=False, debug=False, num_devices=E)

    # Host-side layouts are chosen so every DMA is a large 2D/3D transfer with
    # long contiguous rows (see kernel() for the packing).
    xT = nc.declare_dram_parameter("xT", [128, NHT * cap], bf16, isOutput=False)
    w1 = nc.declare_dram_parameter("w1", [NFCH, 128, NFT * H], bf16, isOutput=False)
    w2 = nc.declare_dram_parameter("w2", [NFCH, 128, NFT * H], bf16, isOutput=False)
    w3 = nc.declare_dram_parameter("w3", [NFCH, 128, NFT * H], bf16, isOutput=False)
    b1 = nc.declare_dram_parameter("b1", [128, F // 128], f32, isOutput=False)
    b3 = nc.declare_dram_parameter("b3", [128, NHT], f32, isOutput=False)
    if use_b2:
        b2 = nc.declare_dram_parameter("b2", [128, F // 128], f32, isOutput=False)
    yT = nc.declare_dram_parameter("yT", [128, NHT * cap], f32, isOutput=True)

    with tile.TileContext(nc) as tc:
        with (
            tc.tile_pool(name="xp", bufs=1) as xp,
            tc.tile_pool(name="yp", bufs=1) as yp,
            tc.tile_pool(name="wp", bufs=2) as wp,
            tc.tile_pool(name="hp", bufs=2) as hp,
            tc.tile_pool(name="sp", bufs=3) as sp,
            tc.tile_pool(name="bp", bufs=1) as bp,
            tc.tile_pool(name="pg", bufs=2, space="PSUM") as pg,
            tc.tile_pool(name="pu", bufs=2, space="PSUM") as pu,
            tc.tile_pool(name="py", bufs=2, space="PSUM") as py,
        ):
            # Biases (tiny, resident)
            b1t = bp.tile([128, F // 128], f32, tag="b1", name="b1t")
            nc.sync.dma_start(b1t[:], b1[:])
            b3t = bp.tile([128, NHT], f32, tag="b3", name="b3t")
            nc.sync.dma_start(b3t[:], b3[:])
            if use_b2:
                b2t = bp.tile([128, F // 128], f32, tag="b2", name="b2t")
                nc.sync.dma_start(b2t[:], b2[:])

            # Tokens (resident, bf16): one [128, NHT*cap] tile in BLOCK-major
            # column order — token block at offset `off` occupies columns
            # [NHT*off, NHT*(off+sz)), h-tile i contiguous inside it. The host
            # supplies the identical layout, so each block is ONE contiguous
            # 2D transfer with multi-KB rows.
            xall = xp.tile([128, NHT * cap], bf16, name="xall")

            def xsl(i, off, sz):  # moving operand [128, sz] for h-tile i
                base = NHT * off + i * sz
                return xall[:, base:base + sz]

            # Output accumulator (resident, f32), same column layout as xall.
            yall = yp.tile([128, NHT * cap], f32, name="yall")

            def ysl(i, off, sz):
                return yall[:, i * cap + off:i * cap + off + sz]

            # Prologue. Token block 0 (one contiguous ~1MB 2D transfer) then
            # the remaining blocks ride the scalar HWDGE queue while the first
            # f-chunk's weights stream on the sync queue in parallel, first w1
            # piece first. The PE starts once block 0 and w1's first piece
            # land (~11us) and never waits again.
            _, sz0 = blocks[0]
            nc.scalar.dma_start(xall[:, 0:NHT * sz0], xT[:, 0:NHT * sz0])
            w1c = wp.tile([128, NFT * H], bf16, tag="w1", name="w1c")
            w2c = wp.tile([128, NFT * H], bf16, tag="w2", name="w2c")
            for j in range(NFT):
                jsl = slice(j * H, (j + 1) * H)
                nc.sync.dma_start(w1c[:, jsl], w1[0][:, jsl])
                nc.scalar.dma_start(w2c[:, jsl], w2[0][:, jsl])
            w3c = wp.tile([128, NFT * H], bf16, tag="w3", name="w3c")
            nc.sync.dma_start(w3c[:], w3[0])
            if len(blocks) > 1:
                # remaining token blocks, in consumption order
                for off, sz in blocks[1:]:
                    lo, hi = NHT * off, NHT * (off + sz)
                    nc.scalar.dma_start(xall[:, lo:hi], xT[:, lo:hi])

            for fc in range(NFCH):
                if fc > 0:
                    # Stream this f-chunk's weights (each byte loaded once).
                    w1c = wp.tile([128, NFT * H], bf16, tag="w1", name="w1c")
                    nc.sync.dma_start(w1c[:], w1[fc])
                    w2c = wp.tile([128, NFT * H], bf16, tag="w2", name="w2c")
                    nc.sync.dma_start(w2c[:], w2[fc])
                    w3c = wp.tile([128, NFT * H], bf16, tag="w3", name="w3c")
                    nc.sync.dma_start(w3c[:], w3[fc])

                def stage_b(off, sz, ht_tiles):
                    # Stage B: y_T[h, tok] += w3_chunk.T @ h_T
                    # w3c columns: (j, h) -> f-tile j, output col h.
                    for i in range(NHT):
                        psy = py.tile([128, sz], f32, tag="y", name="psy")
                        for j in range(NFT):
                            nc.tensor.matmul(
                                psy[:],
                                w3c[:, j * H + i * 128:j * H + (i + 1) * 128],
                                ht_tiles[j][:],
                                start=(j == 0), stop=(j == NFT - 1),
                            )
                        if fc == 0:
                            nc.scalar.activation(
                                ysl(i, off, sz), psy[:], AF.Identity,
                                bias=b3t[:, i:i + 1],
                            )
                        else:
                            nc.vector.tensor_add(
                                ysl(i, off, sz), ysl(i, off, sz), psy[:]
                            )
                    if fc == NFCH - 1:
                        for i in range(NHT):
                            lo, hi = i * cap + off, i * cap + off + sz
                            nc.sync.dma_start(yT[:, lo:hi], yall[:, lo:hi])

                pending = None  # (off, sz, ht_tiles) awaiting stage B
                for off, sz in blocks:
                    # Stage A: h_T[f, tok] = silu(g_T + b1) * (u_T + b2)
                    # w1c/w2c columns: (j, i, q) -> f-tile j, h-tile i, col q.
                    ht_tiles = []
                    for j in range(NFT):
                        fg = fc * NFT + j  # global f-tile index
                        psg = pg.tile([128, sz], f32, tag="g", name="psg")
                        for i in range(NHT):
                            base = (j * NHT + i) * 128
                            nc.tensor.matmul(
                                psg[:], w1c[:, base:base + 128], xsl(i, off, sz),
                                start=(i == 0), stop=(i == NHT - 1),
                            )
                        s = sp.tile([128, sz], f32, tag="s", name="stile")
                        nc.scalar.activation(
                            s[:], psg[:], AF.Silu, bias=b1t[:, fg:fg + 1]
                        )
                        psu = pu.tile([128, sz], f32, tag="u", name="psu")
                        for i in range(NHT):
                            base = (j * NHT + i) * 128
                            nc.tensor.matmul(
                                psu[:], w2c[:, base:base + 128], xsl(i, off, sz),
                                start=(i == 0), stop=(i == NHT - 1),
                            )
                        h = hp.tile([128, sz], bf16, tag=f"h{j}", name=f"htile{j}")
                        if use_b2:
                            u2 = sp.tile([128, sz], f32, tag="u2", name="u2tile")
                            nc.scalar.activation(
                                u2[:], psu[:], AF.Identity, bias=b2t[:, fg:fg + 1]
                            )
                            nc.vector.tensor_mul(h[:], s[:], u2[:])
                        else:
                            nc.vector.tensor_mul(h[:], s[:], psu[:])
                        ht_tiles.append(h)

                    if pending is not None:
                        stage_b(*pending)
                    pending = (off, sz, ht_tiles)
                stage_b(*pending)

    nc.finalize()
    return nc


def _route(x2d: np.ndarray, router_w: np.ndarray):
    """Host router: softmax over experts, top-2. Returns per-expert token
    index lists and combine weights."""
    logits = x2d @ router_w                       # [T, E]
    logits -= logits.max(axis=-1, keepdims=True)
    p = np.exp(logits, dtype=np.float32)
    p /= p.sum(axis=-1, keepdims=True)
    # top-2 expert ids per token (ties: lower index first, like lax.top_k)
    order = np.argsort(-p, axis=-1, kind="stable")[:, :K]   # [T, K]
    idx_e, cw_e = [], []
    for e in range(E):
        sel = np.nonzero((order == e).any(axis=1))[0]
        idx_e.append(sel)
        cw_e.append(p[sel, e])
    return idx_e, cw_e


def _pack_w12(w: np.ndarray) -> np.ndarray:
    """[H, F] f32 -> [NFCH, 128, NFT*NHT*128] bf16 with column order (j, i, q):
    chunk c, partition p, f-tile j, h-tile i, col q = w[i*128+p, c*FCH+j*128+q].
    """
    t = np.asarray(w, dtype=np.float32).reshape(NHT, 128, NFCH, NFT, 128)
    t = t.transpose(2, 1, 3, 0, 4)  # [c, p, j, i, q]
    return np.ascontiguousarray(t.astype(_BF16)).reshape(NFCH, 128, NFT * H)


def _pack_w3(w: np.ndarray) -> np.ndarray:
    """[F, H] f32 -> [NFCH, 128, NFT*H] bf16 with column order (j, h):
    chunk c, partition p (= f within f-tile j) -> w[c*FCH+j*128+p, h]."""
    t = np.asarray(w, dtype=np.float32).reshape(NFCH, NFT, 128, H)
    t = t.transpose(0, 2, 1, 3)  # [c, p, j, h]
    return np.ascontiguousarray(t.astype(_BF16)).reshape(NFCH, 128, NFT * H)


def kernel(x, router_w, w1, b1, w2, b2, w3, b3):
    from concourse.bass_utils import run_bass_kernel_spmd

    B, S, _ = x.shape
    T = B * S
    x2d = np.ascontiguousarray(x, dtype=np.float32).reshape(T, H)

    idx_e, cw_e = _route(x2d, np.asarray(router_w, dtype=np.float32))
    max_n = max(len(i) for i in idx_e)
    blocks, cap = _blocks_for(max_n)

    use_b2 = bool(np.any(b2))
    key = (tuple(blocks), use_b2)
    nc = _kernel_cache.get(key)
    if nc is None:
        nc = _build(blocks, use_b2)
        _kernel_cache[key] = nc

    in_maps = []
    for e in range(E):
        idx = idx_e[e]
        xg = np.zeros((cap, H), dtype=np.float32)
        xg[: len(idx)] = x2d[idx]
        # [cap, H] -> [128, NHT*cap], block-major columns: block at token
        # offset `off` spans cols [NHT*off, NHT*(off+sz)), h-tile i contiguous
        # inside it: col = NHT*off + i*sz + t.
        xb = xg.astype(_BF16)
        xTe = np.concatenate(
            [
                xb[off:off + sz].reshape(sz, NHT, 128)
                .transpose(2, 1, 0).reshape(128, NHT * sz)
                for off, sz in blocks
            ],
            axis=1,
        )
        xTe = np.ascontiguousarray(xTe)
        m = {
            "xT": xTe,
            "w1": _pack_w12(w1[e]),
            "w2": _pack_w12(w2[e]),
            "w3": _pack_w3(w3[e]),
            "b1": np.ascontiguousarray(
                np.asarray(b1[e], dtype=np.float32).reshape(F // 128, 128).T
            ),
            "b3": np.ascontiguousarray(
                np.asarray(b3[e], dtype=np.float32).reshape(NHT, 128).T
            ),
        }
        if use_b2:
            m["b2"] = np.ascontiguousarray(
                np.asarray(b2[e], dtype=np.float32).reshape(F // 128, 128).T
            )
        in_maps.append(m)

    global _last_in_maps
    _last_in_maps = in_maps
    res = run_bass_kernel_spmd(nc, in_maps, core_ids=list(range(E)))

    out = np.zeros((T, H), dtype=np.float32)
    for e in range(E):
        idx = idx_e[e]
        n = len(idx)
        # yT [128, NHT*cap] -> y[t, h]: y[t, i*128+p] = yT[p, i*cap+t]
        yTe = res.results[e]["yT"].reshape(128, NHT, cap)
        ye = yTe[:, :, :n].transpose(2, 1, 0).reshape(n, H)
        out[idx] += ye * cw_e[e][:, None]
    return out.reshape(B, S, H)



# revision 2
# speedup vs baseline: 1.0012x; 1.0012x over previous
"""MoE FFN (top-2 of 8 experts, SwiGLU) for 8 Trainium2 NeuronCores.

Strategy: load-balanced expert parallelism. The router (tiny [T,H]@[H,E]
matmul + softmax + top-2) runs on host as part of sharding; the 16384
(token, expert) pairs are packed into 8 cores x 2 expert-cells of uniform
capacities (c1, c2) found by a small feasibility search, so every core gets
~2048 pairs instead of the max expert load (~2180). Each cell is bound to
one expert; the host supplies that expert's packed weights as the cell's
weight parameters (shared references, no extra packing). Each core runs a
dense SwiGLU FFN over its cells' tokens in bf16 (fp32 PSUM accumulation),
feature-on-partition / token-on-free-dim, weights streamed chunk-by-chunk
(chunk-major over both cells) so SBUF holds one f-chunk per cell turn.

Per-core device program per (f-chunk fc, cell g), blocks of <=512 tokens:
  g_T[f, t] = sum_i w1[h_i, f]^T @ x_T[h_i, t]        (PSUM accum over h-tiles)
  u_T[f, t] likewise with w2
  h_T[f, t] = silu(g_T + b1) * (u_T + b2)             (ACT + DVE, -> bf16)
  y_T[h, t] = sum_f w3[f, h]^T @ h_T[f, t] + b3       (PSUM accum per f-chunk,
                                                       accumulated in SBUF f32)
At the last chunk the accumulated y is emitted as bf16 and written back with
one fused DMA per block on the (otherwise idle) gpsimd SWDGE queue, so the
write-outs never block the weight-streaming queues. A short burst of warm-up
matmuls on a memset tile flips the PE HAM clock-gate to 8/8 before the first
real data lands, and the prologue DMAs are ordered so the first token block
and first w1/w2 pieces arrive as early as possible.
"""

import numpy as np
import ml_dtypes

E = 8       # experts
K = 2       # top-k
H = 1024    # hidden
F = 4096    # ffn dim
BLK = 512   # max tokens per block (moving free dim of every matmul)
FCH = 512   # f-chunk size (weight streaming granularity); FCH % 128 == 0

NHT = H // 128    # h-tiles
NFCH = F // FCH   # f-chunks
NFT = FCH // 128  # f-tiles per chunk

_BF16 = ml_dtypes.bfloat16

_kernel_cache: dict[object, object] = {}
_last_in_maps = None


def _cell_blocks(c: int, small_first: bool):
    """Decompose a cell capacity into token blocks of <=512."""
    r = c % BLK
    blocks = [BLK] * (c // BLK)
    if r:
        blocks = ([r] + blocks) if small_first else (blocks + [r])
    return blocks


def _plan_cells(loads: list[int]):
    """Find uniform cell capacities (c1 >= c2) and an assignment of experts
    to the 8 c1-cells + 8 c2-cells minimizing cap = c1 + c2.

    Returns (c1, c2, cells1, cells2) where cells1/cells2 are length-8 lists
    of (expert, n_tokens) per core (n_tokens may be 0 for unused cells)."""
    order = sorted(range(E), key=lambda e: -loads[e])

    def try_fit(c1, c2):
        # DFS over experts (desc load): pick (a, b) cells with
        # a*c1 + b*c2 >= load, total a <= 8, b <= 8.
        picks = {}

        def dfs(i, a_left, b_left):
            if i == len(order):
                return True
            L = loads[order[i]]
            cands = []
            for a in range(0, a_left + 1):
                rem = L - a * c1
                b = 0 if rem <= 0 else -(-rem // c2)
                if b <= b_left:
                    cands.append((a + b, a, b))
            cands.sort()
            for _, a, b in cands:
                picks[order[i]] = (a, b)
                if dfs(i + 1, a_left - a, b_left - b):
                    return True
            picks.pop(order[i], None)
            return False

        return picks if dfs(0, E, E) else None

    for cap in range(2048, 2048 + 1024, 16):
        lo = (cap + 1) // 2
        lo = -(-lo // 16) * 16
        for c1 in range(lo, cap - 255, 16):
            c2 = cap - c1
            if c2 < 256 or c2 > c1:
                continue
            picks = try_fit(c1, c2)
            if picks is not None:
                # materialize cells: assign expert cells to cores in order
                cells1, cells2 = [], []
                for e in order:
                    a, b = picks[e]
                    rem = loads[e]
                    for _ in range(a):
                        n = min(rem, c1)
                        cells1.append((e, n))
                        rem -= n
                    for _ in range(b):
                        n = min(rem, c2)
                        cells2.append((e, n))
                        rem -= n
                while len(cells1) < E:
                    cells1.append((0, 0))
                while len(cells2) < E:
                    cells2.append((0, 0))
                return c1, c2, cells1, cells2
    raise RuntimeError("no feasible cell plan found")


def _build(c1: int, c2: int, use_b2: bool):
    """Build the per-core Bass/Tile program for cell capacities (c1, c2)."""
    import concourse.bass as bass  # noqa: F401
    import concourse.tile as tile
    from concourse import bacc, mybir

    bf16 = mybir.dt.bfloat16
    f32 = mybir.dt.float32
    AF = mybir.ActivationFunctionType

    cap = c1 + c2
    # blocks as (global_token_offset, size); cell A small-block first (fast
    # prologue ramp), cell B big-first (small tail).
    blocks = []
    off = 0
    for sz in _cell_blocks(c1, small_first=True):
        blocks.append((off, sz, 0))
        off += sz
    for sz in _cell_blocks(c2, small_first=False):
        blocks.append((off, sz, 1))
        off += sz
    assert off == cap
    a_blocks = [(o, s) for o, s, g in blocks if g == 0]

    nc = bacc.Bacc("TRN2", target_bir_lowering=False, debug=False, num_devices=E)

    xT = nc.declare_dram_parameter("xT", [128, NHT * cap], bf16, isOutput=False)
    wps = []  # weight params per group: (w1, w2, w3)
    bps = []  # bias params per group: (b1, b3) or (b1, b2, b3)
    for g in range(2):
        w1 = nc.declare_dram_parameter(f"w1{g}", [NFCH, 128, NFT * H], bf16, isOutput=False)
        w2 = nc.declare_dram_parameter(f"w2{g}", [NFCH, 128, NFT * H], bf16, isOutput=False)
        w3 = nc.declare_dram_parameter(f"w3{g}", [NFCH, 128, NFT * H], bf16, isOutput=False)
        wps.append((w1, w2, w3))
        b1 = nc.declare_dram_parameter(f"b1{g}", [128, F // 128], f32, isOutput=False)
        b3 = nc.declare_dram_parameter(f"b3{g}", [128, NHT], f32, isOutput=False)
        if use_b2:
            b2 = nc.declare_dram_parameter(f"b2{g}", [128, F // 128], f32, isOutput=False)
            bps.append((b1, b2, b3))
        else:
            bps.append((b1, b3))
    yT = nc.declare_dram_parameter("yT", [128, NHT * cap], bf16, isOutput=True)

    with tile.TileContext(nc) as tc:
        with (
            tc.tile_pool(name="xp", bufs=1) as xp,
            tc.tile_pool(name="yp", bufs=1) as yp,
            tc.tile_pool(name="op", bufs=2) as op,
            tc.tile_pool(name="wp", bufs=2) as wp,
            tc.tile_pool(name="hp", bufs=2) as hp,
            tc.tile_pool(name="sp", bufs=3) as sp,
            tc.tile_pool(name="pg", bufs=2, space="PSUM") as pg,
            tc.tile_pool(name="pu", bufs=2, space="PSUM") as pu,
            tc.tile_pool(name="py", bufs=2, space="PSUM") as py,
            tc.tile_pool(name="pw", bufs=1, space="PSUM") as pw,
        ):
            # ---- HAM warm-up: keep the PE busy from ~4us so the clock-gate
            # is at 8/8 when the first real matmul issues. No data deps.
            warm = xp.tile([128, 512], bf16, name="warm")
            nc.vector.memset(warm[:], 0)
            psw = pw.tile([128, 512], f32, name="psw")
            NWARM = 12
            for k in range(NWARM):
                nc.tensor.matmul(
                    psw[:], warm[:, 0:128], warm[:],
                    start=(k == 0), stop=(k == NWARM - 1),
                )

            # ---- resident tiles
            # Tokens (bf16): block-major columns — block at global offset
            # `off` spans cols [NHT*off, NHT*(off+sz)), h-tile i contiguous
            # inside it (col = NHT*off + i*sz + t). Host supplies identical
            # layout: each block is ONE contiguous 2D transfer.
            xall = xp.tile([128, NHT * cap], bf16, name="xall")

            def xsl(i, off, sz):
                base = NHT * off + i * sz
                return xall[:, base:base + sz]

            # f32 accumulator for chunks 0..NFCH-2, h-tile-major columns.
            yall = yp.tile([128, NHT * cap], f32, name="yall")

            def ysl(i, off, sz):
                return yall[:, i * cap + off:i * cap + off + sz]

            # ---- biases (tiny, resident)
            bts = []
            for g in range(2):
                b1t = xp.tile([128, F // 128], f32, name=f"b1t{g}")
                nc.scalar.dma_start(b1t[:], bps[g][0][:])
                b3t = xp.tile([128, NHT], f32, name=f"b3t{g}")
                nc.scalar.dma_start(b3t[:], bps[g][-1][:])
                if use_b2:
                    b2t = xp.tile([128, F // 128], f32, name=f"b2t{g}")
                    nc.scalar.dma_start(b2t[:], bps[g][1][:])
                    bts.append((b1t, b2t, b3t))
                else:
                    bts.append((b1t, b3t))

            # ---- prologue DMAs, critical path first.
            # sync q: w1/w2 j-pieces of (chunk0, cell A) interleaved, then w3.
            # scalar q: cell-A token blocks (small block first), then cell B.
            w1c = wp.tile([128, NFT * H], bf16, tag="w1", name="w1c")
            w2c = wp.tile([128, NFT * H], bf16, tag="w2", name="w2c")
            for j in range(NFT):
                jsl = slice(j * H, (j + 1) * H)
                nc.sync.dma_start(w1c[:, jsl], wps[0][0][0][:, jsl])
                nc.sync.dma_start(w2c[:, jsl], wps[0][1][0][:, jsl])
            w3c = wp.tile([128, NFT * H], bf16, tag="w3", name="w3c")
            nc.sync.dma_start(w3c[:], wps[0][2][0])
            for o, s in a_blocks:
                lo, hi = NHT * o, NHT * (o + s)
                nc.scalar.dma_start(xall[:, lo:hi], xT[:, lo:hi])
            for o, s, g in blocks:
                if g == 1:
                    lo, hi = NHT * o, NHT * (o + s)
                    nc.scalar.dma_start(xall[:, lo:hi], xT[:, lo:hi])

            def stage_b(fc, grp, off, sz, ht_tiles, w3t):
                b3t = bts[grp][-1]
                yo = None
                if fc == NFCH - 1:
                    yo = op.tile([128, NHT * sz], bf16, tag="yo", name="yo")
                for i in range(NHT):
                    psy = py.tile([128, sz], f32, tag="y", name="psy")
                    for j in range(NFT):
                        nc.tensor.matmul(
                            psy[:],
                            w3t[:, j * H + i * 128:j * H + (i + 1) * 128],
                            ht_tiles[j][:],
                            start=(j == 0), stop=(j == NFT - 1),
                        )
                    if fc == 0:
                        nc.scalar.activation(
                            ysl(i, off, sz), psy[:], AF.Identity,
                            bias=b3t[:, i:i + 1],
                        )
                    elif fc < NFCH - 1:
                        nc.vector.tensor_add(
                            ysl(i, off, sz), ysl(i, off, sz), psy[:]
                        )
                    else:
                        nc.vector.tensor_add(
                            yo[:, i * sz:(i + 1) * sz], ysl(i, off, sz), psy[:]
                        )
                if fc == NFCH - 1:
                    # one fused write-out per block on the idle SWDGE queue
                    lo, hi = NHT * off, NHT * (off + sz)
                    nc.gpsimd.dma_start(yT[:, lo:hi], yo[:])

            pending = None
            for fc in range(NFCH):
                for grp in range(2):
                    if fc > 0 or grp > 0:
                        w1c = wp.tile([128, NFT * H], bf16, tag="w1", name="w1c")
                        nc.sync.dma_start(w1c[:], wps[grp][0][fc])
                        w2c = wp.tile([128, NFT * H], bf16, tag="w2", name="w2c")
                        nc.sync.dma_start(w2c[:], wps[grp][1][fc])
                        w3c = wp.tile([128, NFT * H], bf16, tag="w3", name="w3c")
                        nc.sync.dma_start(w3c[:], wps[grp][2][fc])
                    b1t = bts[grp][0]
                    for off, sz, g in blocks:
                        if g != grp:
                            continue
                        # Stage A: h_T[f, tok] = silu(g_T + b1) * (u_T + b2)
                        ht_tiles = []
                        for j in range(NFT):
                            fg = fc * NFT + j
                            psg = pg.tile([128, sz], f32, tag="g", name="psg")
                            for i in range(NHT):
                                base = (j * NHT + i) * 128
                                nc.tensor.matmul(
                                    psg[:], w1c[:, base:base + 128],
                                    xsl(i, off, sz),
                                    start=(i == 0), stop=(i == NHT - 1),
                                )
                            s = sp.tile([128, sz], f32, tag="s", name="stile")
                            nc.scalar.activation(
                                s[:], psg[:], AF.Silu, bias=b1t[:, fg:fg + 1]
                            )
                            psu = pu.tile([128, sz], f32, tag="u", name="psu")
                            for i in range(NHT):
                                base = (j * NHT + i) * 128
                                nc.tensor.matmul(
                                    psu[:], w2c[:, base:base + 128],
                                    xsl(i, off, sz),
                                    start=(i == 0), stop=(i == NHT - 1),
                                )
                            h = hp.tile([128, sz], bf16, tag=f"h{j}", name=f"htile{j}")
                            if use_b2:
                                b2t = bts[grp][1]
                                u2 = sp.tile([128, sz], f32, tag="u2", name="u2tile")
                                nc.scalar.activation(
                                    u2[:], psu[:], AF.Identity,
                                    bias=b2t[:, fg:fg + 1]
                                )
                                nc.vector.tensor_mul(h[:], s[:], u2[:])
                            else:
                                nc.vector.tensor_mul(h[:], s[:], psu[:])
                            ht_tiles.append(h)

                        if pending is not None:
                            stage_b(*pending)
                        pending = (fc, grp, off, sz, ht_tiles, w3c)
            stage_b(*pending)

    nc.finalize()
    return nc


def _route(x2d: np.ndarray, router_w: np.ndarray):
    """Host router: softmax over experts, top-2. Returns per-expert token
    index lists and combine weights."""
    logits = x2d @ router_w                       # [T, E]
    logits -= logits.max(axis=-1, keepdims=True)
    p = np.exp(logits, dtype=np.float32)
    p /= p.sum(axis=-1, keepdims=True)
    order = np.argsort(-p, axis=-1, kind="stable")[:, :K]   # [T, K]
    idx_e, cw_e = [], []
    for e in range(E):
        sel = np.nonzero((order == e).any(axis=1))[0]
        idx_e.append(sel)
        cw_e.append(p[sel, e])
    return idx_e, cw_e


def _pack_w12(w: np.ndarray) -> np.ndarray:
    """[H, F] f32 -> [NFCH, 128, NFT*NHT*128] bf16 with column order (j, i, q):
    chunk c, partition p, f-tile j, h-tile i, col q = w[i*128+p, c*FCH+j*128+q].
    """
    t = np.asarray(w, dtype=np.float32).reshape(NHT, 128, NFCH, NFT, 128)
    t = t.transpose(2, 1, 3, 0, 4)  # [c, p, j, i, q]
    return np.ascontiguousarray(t.astype(_BF16)).reshape(NFCH, 128, NFT * H)


def _pack_w3(w: np.ndarray) -> np.ndarray:
    """[F, H] f32 -> [NFCH, 128, NFT*H] bf16 with column order (j, h):
    chunk c, partition p (= f within f-tile j) -> w[c*FCH+j*128+p, h]."""
    t = np.asarray(w, dtype=np.float32).reshape(NFCH, NFT, 128, H)
    t = t.transpose(0, 2, 1, 3)  # [c, p, j, h]
    return np.ascontiguousarray(t.astype(_BF16)).reshape(NFCH, 128, NFT * H)


def _core_blocks(c1: int, c2: int):
    blocks = []
    off = 0
    for sz in _cell_blocks(c1, small_first=True):
        blocks.append((off, sz, 0))
        off += sz
    for sz in _cell_blocks(c2, small_first=False):
        blocks.append((off, sz, 1))
        off += sz
    return blocks


def kernel(x, router_w, w1, b1, w2, b2, w3, b3):
    from concourse.bass_utils import run_bass_kernel_spmd

    B, S, _ = x.shape
    T = B * S
    x2d = np.ascontiguousarray(x, dtype=np.float32).reshape(T, H)

    idx_e, cw_e = _route(x2d, np.asarray(router_w, dtype=np.float32))
    loads = [len(i) for i in idx_e]
    c1, c2, cells1, cells2 = _plan_cells(loads)
    cap = c1 + c2

    # token ranges per cell: experts consume their index lists in cell order
    # (cells1 scan order, then cells2) — must match _plan_cells's fill order.
    eoff = [0] * E
    core_cells = [[None, None] for _ in range(E)]
    for g, cells, ccap in ((0, cells1, c1), (1, cells2, c2)):
        for core, (e, n) in enumerate(cells):
            core_cells[core][g] = (e, eoff[e], n)
            eoff[e] += n
    for e in range(E):
        assert eoff[e] == loads[e], (e, eoff[e], loads[e])

    use_b2 = bool(np.any(b2))
    key = (c1, c2, use_b2)
    nc = _kernel_cache.get(key)
    if nc is None:
        nc = _build(c1, c2, use_b2)
        _kernel_cache[key] = nc

    # pack weights once per expert (in_maps share references)
    pw1 = [_pack_w12(w1[e]) for e in range(E)]
    pw2 = [_pack_w12(w2[e]) for e in range(E)]
    pw3 = [_pack_w3(w3[e]) for e in range(E)]
    pb1 = [
        np.ascontiguousarray(
            np.asarray(b1[e], dtype=np.float32).reshape(F // 128, 128).T
        )
        for e in range(E)
    ]
    pb3 = [
        np.ascontiguousarray(
            np.asarray(b3[e], dtype=np.float32).reshape(NHT, 128).T
        )
        for e in range(E)
    ]
    if use_b2:
        pb2 = [
            np.ascontiguousarray(
                np.asarray(b2[e], dtype=np.float32).reshape(F // 128, 128).T
            )
            for e in range(E)
        ]

    blocks = _core_blocks(c1, c2)
    cell_off = (0, c1)

    in_maps = []
    for core in range(E):
        # gather this core's tokens: cell A rows [0, c1), cell B rows [c1, cap)
        xg = np.zeros((cap, H), dtype=np.float32)
        for g in range(2):
            e, st, n = core_cells[core][g]
            if n:
                xg[cell_off[g]:cell_off[g] + n] = x2d[idx_e[e][st:st + n]]
        xb = xg.astype(_BF16)
        xTe = np.concatenate(
            [
                xb[off:off + sz].reshape(sz, NHT, 128)
                .transpose(2, 1, 0).reshape(128, NHT * sz)
                for off, sz, _ in blocks
            ],
            axis=1,
        )
        m = {"xT": np.ascontiguousarray(xTe)}
        for g in range(2):
            e = core_cells[core][g][0]
            m[f"w1{g}"] = pw1[e]
            m[f"w2{g}"] = pw2[e]
            m[f"w3{g}"] = pw3[e]
            m[f"b1{g}"] = pb1[e]
            m[f"b3{g}"] = pb3[e]
            if use_b2:
                m[f"b2{g}"] = pb2[e]
        in_maps.append(m)

    global _last_in_maps
    _last_in_maps = in_maps
    res = run_bass_kernel_spmd(nc, in_maps, core_ids=list(range(E)))

    out = np.zeros((T, H), dtype=np.float32)
    for core in range(E):
        yTe = np.asarray(res.results[core]["yT"], dtype=np.float32)
        for g in range(2):
            e, st, n = core_cells[core][g]
            if not n:
                continue
            co = cell_off[g]
            # per-block unpack: cols NHT*off + i*sz + t
            ye = np.empty((core_cells[core][g][2], H), dtype=np.float32)
            for off, sz, bg in blocks:
                if bg != g:
                    continue
                rel = off - co   # row range of this block within the cell
                if rel >= n:
                    continue
                take = min(sz, n - rel)
                blk = yTe[:, NHT * off:NHT * (off + sz)].reshape(128, NHT, sz)
                ye[rel:rel + take] = (
                    blk[:, :, :take].transpose(2, 1, 0).reshape(take, H)
                )
            idx = idx_e[e][st:st + n]
            out[idx] += ye * cw_e[e][st:st + n][:, None]
    return out.reshape(B, S, H)


# revision 6
# speedup vs baseline: 1.0431x; 1.0418x over previous
"""MoE FFN (top-2 of 8 experts, SwiGLU) for 8 Trainium2 NeuronCores.

Strategy: load-balanced expert parallelism. The router (tiny [T,H]@[H,E]
matmul + softmax + top-2) runs on host as part of sharding; the 16384
(token, expert) pairs are packed into 8 cores x 2 expert-cells of uniform
capacities (c1, c2) found by a small feasibility search, so every core gets
~2048 pairs instead of the max expert load (~2180). Each cell is bound to
one expert; the host supplies that expert's packed weights as the cell's
weight parameters (shared references, no extra packing). Each core runs a
dense SwiGLU FFN over its cells' tokens in bf16 (fp32 PSUM accumulation),
feature-on-partition / token-on-free-dim, weights streamed chunk-by-chunk
(chunk-major over both cells) so SBUF holds one f-chunk per cell turn.

Per-core device program per (f-chunk fc, cell g), blocks of <=512 tokens:
  g_T[f, t] = sum_i w1[h_i, f]^T @ x_T[h_i, t]        (PSUM accum over h-tiles)
  u_T[f, t] likewise with w2
  h_T[f, t] = silu(g_T + b1) * (u_T + b2)             (ACT + DVE, -> bf16)
  y_T[h, t] = sum_f w3[f, h]^T @ h_T[f, t] + b3       (PSUM accum per f-chunk,
                                                       accumulated in SBUF f32)
At the last chunk the accumulated y is emitted as bf16 and written back with
one fused DMA per block on the (otherwise idle) gpsimd SWDGE queue, so the
write-outs never block the weight-streaming queues. A short burst of warm-up
matmuls on a memset tile flips the PE HAM clock-gate to 8/8 before the first
real data lands, and the prologue DMAs are ordered so the first token block
and first w1/w2 pieces arrive as early as possible.
"""

import numpy as np
import ml_dtypes

E = 8       # experts
K = 2       # top-k
H = 1024    # hidden
F = 4096    # ffn dim
BLK = 512   # max tokens per block (moving free dim of every matmul)
FCH = 512   # f-chunk size (weight streaming granularity); FCH % 128 == 0

NHT = H // 128    # h-tiles
NFCH = F // FCH   # f-chunks
NFT = FCH // 128  # f-tiles per chunk

_BF16 = ml_dtypes.bfloat16

_kernel_cache: dict[object, object] = {}
_last_in_maps = None


def _cell_blocks(c: int, small_first: bool):
    """Decompose a cell capacity into token blocks of <=512."""
    r = c % BLK
    blocks = [BLK] * (c // BLK)
    if r:
        blocks = ([r] + blocks) if small_first else (blocks + [r])
    return blocks


def _plan_cells(loads: list[int]):
    """Find uniform cell capacities (c1 >= c2) and an assignment of experts
    to the 8 c1-cells + 8 c2-cells minimizing cap = c1 + c2.

    Returns (c1, c2, cells1, cells2) where cells1/cells2 are length-8 lists
    of (expert, n_tokens) per core (n_tokens may be 0 for unused cells)."""
    order = sorted(range(E), key=lambda e: -loads[e])

    def try_fit(c1, c2):
        # DFS over experts (desc load): pick (a, b) cells with
        # a*c1 + b*c2 >= load, total a <= 8, b <= 8.
        picks = {}

        def dfs(i, a_left, b_left):
            if i == len(order):
                return True
            L = loads[order[i]]
            cands = []
            for a in range(0, a_left + 1):
                rem = L - a * c1
                b = 0 if rem <= 0 else -(-rem // c2)
                if b <= b_left:
                    cands.append((a + b, a, b))
            cands.sort()
            for _, a, b in cands:
                picks[order[i]] = (a, b)
                if dfs(i + 1, a_left - a, b_left - b):
                    return True
            picks.pop(order[i], None)
            return False

        return picks if dfs(0, E, E) else None

    for cap in range(2048, 2048 + 1024, 16):
        lo = (cap + 1) // 2
        lo = -(-lo // 16) * 16
        for c1 in range(lo, cap - 255, 16):
            c2 = cap - c1
            if c2 < 256 or c2 > c1:
                continue
            picks = try_fit(c1, c2)
            if picks is not None:
                # materialize cells: assign expert cells to cores in order
                cells1, cells2 = [], []
                for e in order:
                    a, b = picks[e]
                    rem = loads[e]
                    for _ in range(a):
                        n = min(rem, c1)
                        cells1.append((e, n))
                        rem -= n
                    for _ in range(b):
                        n = min(rem, c2)
                        cells2.append((e, n))
                        rem -= n
                while len(cells1) < E:
                    cells1.append((0, 0))
                while len(cells2) < E:
                    cells2.append((0, 0))
                return c1, c2, cells1, cells2
    raise RuntimeError("no feasible cell plan found")


def _build(c1: int, c2: int, use_b2: bool):
    """Build the per-core Bass/Tile program for cell capacities (c1, c2)."""
    import concourse.bass as bass  # noqa: F401
    import concourse.tile as tile
    from concourse import bacc, mybir

    bf16 = mybir.dt.bfloat16
    f32 = mybir.dt.float32
    AF = mybir.ActivationFunctionType

    cap = c1 + c2
    blocks = _core_blocks(c1, c2)

    nc = bacc.Bacc("TRN2", target_bir_lowering=False, debug=False, num_devices=E)

    xT = nc.declare_dram_parameter("xT", [128, NHT * cap], bf16, isOutput=False)
    wps = []  # weight params per group: (w1, w2, w3)
    bps = []  # bias params per group: (b1, b3) or (b1, b2, b3)
    for g in range(2):
        w1 = nc.declare_dram_parameter(f"w1{g}", [NFCH, 128, NFT * H], bf16, isOutput=False)
        w2 = nc.declare_dram_parameter(f"w2{g}", [NFCH, 128, NFT * H], bf16, isOutput=False)
        w3 = nc.declare_dram_parameter(f"w3{g}", [NFCH, 128, NFT * H], bf16, isOutput=False)
        wps.append((w1, w2, w3))
        b1 = nc.declare_dram_parameter(f"b1{g}", [128, F // 128], f32, isOutput=False)
        b3 = nc.declare_dram_parameter(f"b3{g}", [128, NHT], f32, isOutput=False)
        if use_b2:
            b2 = nc.declare_dram_parameter(f"b2{g}", [128, F // 128], f32, isOutput=False)
            bps.append((b1, b2, b3))
        else:
            bps.append((b1, b3))
    yT = nc.declare_dram_parameter("yT", [128, NHT * cap], bf16, isOutput=True)

    with tile.TileContext(nc) as tc:
        with (
            tc.tile_pool(name="xp", bufs=1) as xp,
            tc.tile_pool(name="yp", bufs=1) as yp,
            tc.tile_pool(name="op", bufs=2) as op,
            tc.tile_pool(name="wp", bufs=2) as wp,
            tc.tile_pool(name="hp", bufs=2) as hp,
            tc.tile_pool(name="sp", bufs=3) as sp,
            tc.tile_pool(name="pg", bufs=2, space="PSUM") as pg,
            tc.tile_pool(name="pu", bufs=2, space="PSUM") as pu,
            tc.tile_pool(name="py", bufs=2, space="PSUM") as py,
            tc.tile_pool(name="pw", bufs=1, space="PSUM") as pw,
        ):
            # ---- HAM warm-up: keep the PE busy from ~4us so the clock-gate
            # is at 8/8 when the first real matmul issues. No data deps.
            warm = xp.tile([128, 512], bf16, name="warm")
            nc.vector.memset(warm[:], 0)
            psw = pw.tile([128, 512], f32, name="psw")
            NWARM = 12
            for k in range(NWARM):
                nc.tensor.matmul(
                    psw[:], warm[:, 0:128], warm[:],
                    start=(k == 0), stop=(k == NWARM - 1),
                )

            # ---- resident tiles
            # Tokens (bf16): block-major columns — block at global offset
            # `off` spans cols [NHT*off, NHT*(off+sz)), h-tile i contiguous
            # inside it (col = NHT*off + i*sz + t). Host supplies identical
            # layout: each block is ONE contiguous 2D transfer.
            xall = xp.tile([128, NHT * cap], bf16, name="xall")

            def xsl(i, off, sz):
                base = NHT * off + i * sz
                return xall[:, base:base + sz]

            # f32 accumulator for chunks 0..NFCH-2, h-tile-major columns.
            yall = yp.tile([128, NHT * cap], f32, name="yall")

            def ysl(i, off, sz):
                return yall[:, i * cap + off:i * cap + off + sz]

            # ---- prologue DMAs, critical path first.
            # scalar q: token blocks in processing order (x of first block is
            # the critical path together with w1/w2 chunk-0 pieces on sync).
            for o, s, g in blocks:
                lo, hi = NHT * o, NHT * (o + s)
                nc.scalar.dma_start(xall[:, lo:hi], xT[:, lo:hi])

            # sync q: w1/w2 j-pieces of (chunk0, cell A) interleaved, then
            # biases, then w3.
            bts = [None, None]
            w1cA = wp.tile([128, NFT * H], bf16, tag="w1", name="w1c")
            w2cA = wp.tile([128, NFT * H], bf16, tag="w2", name="w2c")
            for j in range(NFT):
                jsl = slice(j * H, (j + 1) * H)
                nc.sync.dma_start(w1cA[:, jsl], wps[0][0][0][:, jsl])
                nc.sync.dma_start(w2cA[:, jsl], wps[0][1][0][:, jsl])
                if j == 1:
                    b1t = xp.tile([128, F // 128], f32, name="b1t0")
                    nc.sync.dma_start(b1t[:], bps[0][0][:])
                    b3t = xp.tile([128, NHT], f32, name="b3t0")
                    nc.sync.dma_start(b3t[:], bps[0][-1][:])
                    if use_b2:
                        b2t = xp.tile([128, F // 128], f32, name="b2t0")
                        nc.sync.dma_start(b2t[:], bps[0][1][:])
                        bts[0] = (b1t, b2t, b3t)
                    else:
                        bts[0] = (b1t, b3t)
            w3cA = wp.tile([128, NFT * H], bf16, tag="w3", name="w3c")
            nc.sync.dma_start(w3cA[:], wps[0][2][0])
            # cell-B biases
            b1t = xp.tile([128, F // 128], f32, name="b1t1")
            nc.sync.dma_start(b1t[:], bps[1][0][:])
            b3t = xp.tile([128, NHT], f32, name="b3t1")
            nc.sync.dma_start(b3t[:], bps[1][-1][:])
            if use_b2:
                b2t = xp.tile([128, F // 128], f32, name="b2t1")
                nc.sync.dma_start(b2t[:], bps[1][1][:])
                bts[1] = (b1t, b2t, b3t)
            else:
                bts[1] = (b1t, b3t)

            def stage_b(fc, grp, off, sz, ht_tiles, w3t):
                b3t = bts[grp][-1]
                yo = None
                if fc == NFCH - 1:
                    yo = op.tile([128, NHT * sz], bf16, tag="yo", name="yo")
                for i in range(NHT):
                    psy = py.tile([128, sz], f32, tag="y", name="psy")
                    for j in range(NFT):
                        nc.tensor.matmul(
                            psy[:],
                            w3t[:, j * H + i * 128:j * H + (i + 1) * 128],
                            ht_tiles[j][:],
                            start=(j == 0), stop=(j == NFT - 1),
                        )
                    if fc == 0:
                        nc.scalar.activation(
                            ysl(i, off, sz), psy[:], AF.Identity,
                            bias=b3t[:, i:i + 1],
                        )
                    elif fc < NFCH - 1:
                        nc.vector.tensor_add(
                            ysl(i, off, sz), ysl(i, off, sz), psy[:]
                        )
                    else:
                        nc.vector.tensor_add(
                            yo[:, i * sz:(i + 1) * sz], ysl(i, off, sz), psy[:]
                        )
                if fc == NFCH - 1:
                    # one fused write-out per block; sync queue is safe here
                    # because both groups' weight loads for this chunk were
                    # issued before any write-out (no queue-order cycle).
                    lo, hi = NHT * off, NHT * (off + sz)
                    nc.sync.dma_start(yT[:, lo:hi], yo[:])

            pending = None
            for fc in range(NFCH):
                # load this chunk's weights for BOTH cells up front (tag
                # alternation keeps the bufs=2 double-buffering intact)
                wt = [None, None]
                wt[0] = (w1cA, w2cA, w3cA) if fc == 0 else None
                if wt[0] is None:
                    w1c = wp.tile([128, NFT * H], bf16, tag="w1", name="w1c")
                    nc.sync.dma_start(w1c[:], wps[0][0][fc])
                    w2c = wp.tile([128, NFT * H], bf16, tag="w2", name="w2c")
                    nc.sync.dma_start(w2c[:], wps[0][1][fc])
                    w3c = wp.tile([128, NFT * H], bf16, tag="w3", name="w3c")
                    nc.sync.dma_start(w3c[:], wps[0][2][fc])
                    wt[0] = (w1c, w2c, w3c)
                w1c = wp.tile([128, NFT * H], bf16, tag="w1", name="w1c")
                nc.sync.dma_start(w1c[:], wps[1][0][fc])
                w2c = wp.tile([128, NFT * H], bf16, tag="w2", name="w2c")
                nc.sync.dma_start(w2c[:], wps[1][1][fc])
                w3c = wp.tile([128, NFT * H], bf16, tag="w3", name="w3c")
                nc.sync.dma_start(w3c[:], wps[1][2][fc])
                wt[1] = (w1c, w2c, w3c)

                for grp in range(2):
                    w1c, w2c, w3c = wt[grp]
                    b1t = bts[grp][0]
                    for off, sz, g in blocks:
                        if g != grp:
                            continue
                        # Stage A: h_T[f, tok] = silu(g_T + b1) * (u_T + b2)
                        ht_tiles = []
                        for j in range(NFT):
                            fg = fc * NFT + j
                            psg = pg.tile([128, sz], f32, tag="g", name="psg")
                            for i in range(NHT):
                                base = (j * NHT + i) * 128
                                nc.tensor.matmul(
                                    psg[:], w1c[:, base:base + 128],
                                    xsl(i, off, sz),
                                    start=(i == 0), stop=(i == NHT - 1),
                                )
                            s = sp.tile([128, sz], f32, tag="s", name="stile")
                            nc.scalar.activation(
                                s[:], psg[:], AF.Silu, bias=b1t[:, fg:fg + 1]
                            )
                            psu = pu.tile([128, sz], f32, tag="u", name="psu")
                            for i in range(NHT):
                                base = (j * NHT + i) * 128
                                nc.tensor.matmul(
                                    psu[:], w2c[:, base:base + 128],
                                    xsl(i, off, sz),
                                    start=(i == 0), stop=(i == NHT - 1),
                                )
                            h = hp.tile([128, sz], bf16, tag=f"h{j}", name=f"htile{j}")
                            if use_b2:
                                b2t = bts[grp][1]
                                u2 = sp.tile([128, sz], f32, tag="u2", name="u2tile")
                                nc.scalar.activation(
                                    u2[:], psu[:], AF.Identity,
                                    bias=b2t[:, fg:fg + 1]
                                )
                                nc.vector.tensor_mul(h[:], s[:], u2[:])
                            else:
                                nc.vector.tensor_mul(h[:], s[:], psu[:])
                            ht_tiles.append(h)

                        if pending is not None:
                            stage_b(*pending)
                        pending = (fc, grp, off, sz, ht_tiles, w3c)
            stage_b(*pending)

    nc.finalize()
    return nc


def _route(x2d: np.ndarray, router_w: np.ndarray):
    """Host router: softmax over experts, top-2. Returns per-expert token
    index lists and combine weights."""
    logits = x2d @ router_w                       # [T, E]
    logits -= logits.max(axis=-1, keepdims=True)
    p = np.exp(logits, dtype=np.float32)
    p /= p.sum(axis=-1, keepdims=True)
    order = np.argsort(-p, axis=-1, kind="stable")[:, :K]   # [T, K]
    idx_e, cw_e = [], []
    for e in range(E):
        sel = np.nonzero((order == e).any(axis=1))[0]
        idx_e.append(sel)
        cw_e.append(p[sel, e])
    return idx_e, cw_e


def _pack_w12(w: np.ndarray) -> np.ndarray:
    """[H, F] f32 -> [NFCH, 128, NFT*NHT*128] bf16 with column order (j, i, q):
    chunk c, partition p, f-tile j, h-tile i, col q = w[i*128+p, c*FCH+j*128+q].
    """
    t = np.asarray(w, dtype=np.float32).reshape(NHT, 128, NFCH, NFT, 128)
    t = t.transpose(2, 1, 3, 0, 4)  # [c, p, j, i, q]
    return np.ascontiguousarray(t.astype(_BF16)).reshape(NFCH, 128, NFT * H)


def _pack_w3(w: np.ndarray) -> np.ndarray:
    """[F, H] f32 -> [NFCH, 128, NFT*H] bf16 with column order (j, h):
    chunk c, partition p (= f within f-tile j) -> w[c*FCH+j*128+p, h]."""
    t = np.asarray(w, dtype=np.float32).reshape(NFCH, NFT, 128, H)
    t = t.transpose(0, 2, 1, 3)  # [c, p, j, h]
    return np.ascontiguousarray(t.astype(_BF16)).reshape(NFCH, 128, NFT * H)


def _core_blocks(c1: int, c2: int):
    # big blocks first in each cell: the first block's x transfer is on the
    # prologue critical path and a full 512-block keeps the PE streaming.
    blocks = []
    off = 0
    for sz in _cell_blocks(c1, small_first=False):
        blocks.append((off, sz, 0))
        off += sz
    for sz in _cell_blocks(c2, small_first=False):
        blocks.append((off, sz, 1))
        off += sz
    return blocks


def kernel(x, router_w, w1, b1, w2, b2, w3, b3):
    from concourse.bass_utils import run_bass_kernel_spmd

    B, S, _ = x.shape
    T = B * S
    x2d = np.ascontiguousarray(x, dtype=np.float32).reshape(T, H)

    idx_e, cw_e = _route(x2d, np.asarray(router_w, dtype=np.float32))
    loads = [len(i) for i in idx_e]
    c1, c2, cells1, cells2 = _plan_cells(loads)
    cap = c1 + c2

    # token ranges per cell: experts consume their index lists in cell order
    # (cells1 scan order, then cells2) — must match _plan_cells's fill order.
    eoff = [0] * E
    core_cells = [[None, None] for _ in range(E)]
    for g, cells, ccap in ((0, cells1, c1), (1, cells2, c2)):
        for core, (e, n) in enumerate(cells):
            core_cells[core][g] = (e, eoff[e], n)
            eoff[e] += n
    for e in range(E):
        assert eoff[e] == loads[e], (e, eoff[e], loads[e])

    use_b2 = bool(np.any(b2))
    key = (c1, c2, use_b2)
    nc = _kernel_cache.get(key)
    if nc is None:
        nc = _build(c1, c2, use_b2)
        _kernel_cache[key] = nc

    # pack weights once per expert (in_maps share references)
    pw1 = [_pack_w12(w1[e]) for e in range(E)]
    pw2 = [_pack_w12(w2[e]) for e in range(E)]
    pw3 = [_pack_w3(w3[e]) for e in range(E)]
    pb1 = [
        np.ascontiguousarray(
            np.asarray(b1[e], dtype=np.float32).reshape(F // 128, 128).T
        )
        for e in range(E)
    ]
    pb3 = [
        np.ascontiguousarray(
            np.asarray(b3[e], dtype=np.float32).reshape(NHT, 128).T
        )
        for e in range(E)
    ]
    if use_b2:
        pb2 = [
            np.ascontiguousarray(
                np.asarray(b2[e], dtype=np.float32).reshape(F // 128, 128).T
            )
            for e in range(E)
        ]

    blocks = _core_blocks(c1, c2)
    cell_off = (0, c1)

    in_maps = []
    for core in range(E):
        # gather this core's tokens: cell A rows [0, c1), cell B rows [c1, cap)
        xg = np.zeros((cap, H), dtype=np.float32)
        for g in range(2):
            e, st, n = core_cells[core][g]
            if n:
                xg[cell_off[g]:cell_off[g] + n] = x2d[idx_e[e][st:st + n]]
        xb = xg.astype(_BF16)
        xTe = np.concatenate(
            [
                xb[off:off + sz].reshape(sz, NHT, 128)
                .transpose(2, 1, 0).reshape(128, NHT * sz)
                for off, sz, _ in blocks
            ],
            axis=1,
        )
        m = {"xT": np.ascontiguousarray(xTe)}
        for g in range(2):
            e = core_cells[core][g][0]
            m[f"w1{g}"] = pw1[e]
            m[f"w2{g}"] = pw2[e]
            m[f"w3{g}"] = pw3[e]
            m[f"b1{g}"] = pb1[e]
            m[f"b3{g}"] = pb3[e]
            if use_b2:
                m[f"b2{g}"] = pb2[e]
        in_maps.append(m)

    global _last_in_maps
    _last_in_maps = in_maps
    res = run_bass_kernel_spmd(nc, in_maps, core_ids=list(range(E)))

    out = np.zeros((T, H), dtype=np.float32)
    for core in range(E):
        yTe = np.asarray(res.results[core]["yT"], dtype=np.float32)
        for g in range(2):
            e, st, n = core_cells[core][g]
            if not n:
                continue
            co = cell_off[g]
            # per-block unpack: cols NHT*off + i*sz + t
            ye = np.empty((core_cells[core][g][2], H), dtype=np.float32)
            for off, sz, bg in blocks:
                if bg != g:
                    continue
                rel = off - co   # row range of this block within the cell
                if rel >= n:
                    continue
                take = min(sz, n - rel)
                blk = yTe[:, NHT * off:NHT * (off + sz)].reshape(128, NHT, sz)
                ye[rel:rel + take] = (
                    blk[:, :, :take].transpose(2, 1, 0).reshape(take, H)
                )
            idx = idx_e[e][st:st + n]
            out[idx] += ye * cw_e[e][st:st + n][:, None]
    return out.reshape(B, S, H)
